# revision 24
# baseline (speedup 1.0000x reference)
"""Depthwise 3x3 blur of |x| on 8 trn2 NeuronCores (pure data-parallel on batch).

out[n,c] = corr2d(|x[n,c]|, w3x3, pad=1)  with w3x3 = weight[c,0] (same for all c).

Per-core plan (core i owns batch i: [16, 1024, 1024] f32):
  x is host-padded with one zero row/column on every side -> [C, 1026, 1026].
  Each channel is processed as 9 row-tiles: 8 tiles of 126 output rows plus a
  16-row tail. A tile's 128 padded input rows land in SBUF partitions
  (partition = image row), |.| runs on ScalarE (casting to the matmul dtype),
  and the conv is 3 column-shifted banded matmuls per 512-wide PSUM bank on
  TensorE: matmul j applies kernel column j vertically via a banded
  lhsT[k, m] = w3x3[k-m, j], while the +-1 horizontal shift comes from
  offsetting the rhs column window over the padded tile (pad columns supply
  the horizontal zero padding, pad rows the vertical). PSUM (fp32) is evicted
  on ScalarE/VectorE and DMA'd back.

  DMA: 4 row-tiles are loaded per dma_start (2 MiB, overlapping 128-row
  chunks at stride 126 via a raw access pattern) on the Sync HWDGE queue, and
  4 output tiles are stored per dma_start (2 MiB) alternating between the
  GpSimd SWDGE and Scalar HWDGE queues, so loads and stores run on
  independent DMA queues and per-transfer completion bubbles overlap.

  Measured on 8 axon trn2 cores: ~386 us HW exec (HBM roofline ~375 us,
  136 MB/core at ~358 GB/s), relative error ~3e-4 (fp16 input rounding;
  all kernel weights are exact in fp16, accumulation is fp32 in PSUM).
"""

import numpy as np

import concourse.mybir as mybir
from concourse.ap import AP
from concourse import bacc
from concourse.bass import MemorySpace
from concourse.bass_utils import run_bass_kernel_spmd
from concourse.tile import TileContext

N, C, H, W = 8, 16, 1024, 1024
P = 128  # SBUF partitions
MI = 126  # out rows per regular tile
BANK = 512  # fp32 elements per PSUM bank
HP, WP = H + 2, W + 2  # padded image dims
F32 = mybir.dt.float32

DTYPE = "fp16"  # matmul operand dtype: "fp16", "bf16", or "f32r"


def _mm_dt():
    return {
        "fp16": mybir.dt.float16,
        "bf16": mybir.dt.bfloat16,
        "f32r": mybir.dt.float32r,
    }[DTYPE]


def _build_bands(w3x3: np.ndarray) -> np.ndarray:
    """[3, 128, 128] f32 banded lhsT: B[j][k, m] = w3x3[k - m, j]."""
    bands = np.zeros((3, P, P), np.float32)
    for j in range(3):
        for d in range(3):
            for m in range(MI):
                if m + d < P:
                    bands[j, m + d, m] = w3x3[d, j]
    return bands


def _matmuls(nc, ps, bt, at, at_col0, K):
    """3 column-shifted banded matmuls per 512-wide PSUM bank of ps."""
    nbank = ps.shape[1] // BANK
    for b in range(nbank):
        c0 = BANK * b
        for i, j in enumerate((1, 0, 2)):
            nc.tensor.matmul(
                ps[:, c0 : c0 + BANK],
                bt[:K, P * j : P * (j + 1)],
                at[:K, at_col0 + c0 + j : at_col0 + c0 + j + BANK],
                start=(i == 0),
                stop=(i == 2),
            )


def _gen_program():
    mmdt = _mm_dt()
    nc = bacc.Bacc("TRN2", target_bir_lowering=False, debug=False, num_devices=N)

    x = nc.dram_tensor("x", [C, HP, WP], F32, kind="ExternalInput")
    bands = nc.dram_tensor("bands", [3, P, P], mmdt, kind="ExternalInput")
    out = nc.dram_tensor("out", [C, H, W], F32, kind="ExternalOutput")

    with TileContext(nc) as tc:
        with (
            tc.tile_pool(name="consts", bufs=1) as cpool,
            tc.tile_pool(name="xin", bufs=4) as xpool,
            tc.tile_pool(name="xabs", bufs=4) as apool,
            tc.tile_pool(name="oev", bufs=4) as opool,
            tc.tile_pool(name="ps", bufs=3, space=MemorySpace.PSUM) as pspool,
        ):
            # bands go on the gpsimd queue (idle early) so the first x load
            # is the very first transfer on the sync queue
            bt = cpool.tile([P, 3 * P], mmdt)
            for j in range(3):
                nc.gpsimd.dma_start(out=bt[:, P * j : P * (j + 1)], in_=bands[j])

            for c in range(C):
                for q in range(2):  # quads of 4 row-tiles: t = 4q + k
                    r0 = 504 * q  # padded row of chunk 0
                    xt = xpool.tile([P, 4 * WP], F32)
                    src = AP(
                        x, c * HP * WP + r0 * WP,
                        [[WP, P], [MI * WP, 4], [1, WP]],
                    )
                    nc.sync.dma_start(out=xt[:], in_=src)

                    at = apool.tile([P, 4 * WP], mmdt)
                    nc.scalar.activation(
                        at[:], xt[:], mybir.ActivationFunctionType.Abs
                    )

                    ot = opool.tile([P, 4 * W], F32)
                    for k in range(4):
                        ps = pspool.tile([P, W], F32)
                        _matmuls(nc, ps, bt, at, k * WP, P)
                        ev = nc.vector.tensor_copy if k % 2 else nc.scalar.copy
                        ev(ot[:MI, k * W : (k + 1) * W], ps[:MI])

                    dst = AP(
                        out, c * H * W + 4 * MI * q * W,
                        [[W, MI], [MI * W, 4], [1, W]],
                    )
                    stq = nc.gpsimd if (2 * c + q) % 2 == 0 else nc.scalar
                    stq.dma_start(out=dst, in_=ot[:MI, :])

                # tail: out rows 1008..1023 (M=16), padded rhs rows 1008..1025
                K8, M8 = 18, 16
                xt = xpool.tile([P, 4 * WP], F32)
                nc.sync.dma_start(out=xt[:K8, :WP], in_=x[c, 1008 : 1008 + K8])
                at = apool.tile([P, 4 * WP], mmdt)
                nc.scalar.activation(
                    at[:K8, :WP], xt[:K8, :WP], mybir.ActivationFunctionType.Abs
                )
                ps = pspool.tile([P, W], F32)
                _matmuls(nc, ps, bt, at, 0, K8)
                ot = opool.tile([P, 4 * W], F32)
                nc.vector.tensor_copy(ot[:M8, :W], ps[:M8])
                nc.gpsimd.dma_start(out=out[c, 8 * MI :], in_=ot[:M8, :W])

    nc.compile()
    return nc


_PROGRAM = None


def _get_program():
    global _PROGRAM
    if _PROGRAM is None:
        _PROGRAM = _gen_program()
    return _PROGRAM


def _run(x: np.ndarray, weight: np.ndarray, trace: bool = False, tmpdir=None):
    assert x.shape == (N, C, H, W), x.shape
    w3x3 = np.asarray(weight, np.float32)[0, 0]
    np_mmdt = mybir.dt.np(_mm_dt())
    bands = _build_bands(w3x3).astype(np_mmdt)

    xp = np.pad(np.asarray(x, np.float32), ((0, 0), (0, 0), (1, 1), (1, 1)))

    nc = _get_program()
    in_maps = [
        {"x": np.ascontiguousarray(xp[i]), "bands": bands} for i in range(N)
    ]
    res = run_bass_kernel_spmd(
        nc, in_maps, core_ids=list(range(N)), trace=trace, tmpdir=tmpdir
    )
    out = np.stack([res.results[i]["out"] for i in range(N)])
    return out, res


def kernel(x: np.ndarray, weight: np.ndarray) -> np.ndarray:
    out, _ = _run(np.asarray(x), np.asarray(weight))
    return out


# revision 25
# speedup vs baseline: 1.1844x; 1.1844x over previous
"""Depthwise 3x3 blur of |x| on 8 trn2 NeuronCores (pure data-parallel on batch).

out[n,c] = corr2d(|x[n,c]|, w3x3, pad=1)  with w3x3 = weight[c,0] (same for all c).

Per-core plan (core i owns batch i: [16, 1024, 1024] f32):
  x is host-padded with one zero row/column on every side -> [C, 1026, 1026].
  Each channel is processed as 9 row-tiles: 8 tiles of 126 output rows plus a
  16-row tail. A tile's 128 padded input rows land in SBUF partitions
  (partition = image row), |.| runs on ScalarE (casting to the matmul dtype),
  and the conv is 3 column-shifted banded matmuls per 512-wide PSUM bank on
  TensorE: matmul j applies kernel column j vertically via a banded
  lhsT[k, m] = w3x3[k-m, j], while the +-1 horizontal shift comes from
  offsetting the rhs column window over the padded tile (pad columns supply
  the horizontal zero padding, pad rows the vertical). PSUM (fp32) is evicted
  on ScalarE/VectorE and DMA'd back.

  DMA: 4 row-tiles are loaded per dma_start (2 MiB, overlapping 128-row
  chunks at stride 126 via a raw access pattern) on the Sync HWDGE queue, and
  4 output tiles are stored per dma_start (2 MiB) alternating between the
  GpSimd SWDGE and Scalar HWDGE queues, so loads and stores run on
  independent DMA queues and per-transfer completion bubbles overlap.

  Measured on 8 axon trn2 cores: ~386 us HW exec (HBM roofline ~375 us,
  136 MB/core at ~358 GB/s), relative error ~3e-4 (fp16 input rounding;
  all kernel weights are exact in fp16, accumulation is fp32 in PSUM).
"""

import numpy as np

import concourse.mybir as mybir
from concourse.ap import AP
from concourse import bacc
from concourse.bass import MemorySpace
from concourse.bass_utils import run_bass_kernel_spmd
from concourse.tile import TileContext

N, C, H, W = 8, 16, 1024, 1024
P = 128  # SBUF partitions
MI = 126  # out rows per regular tile
BANK = 512  # fp32 elements per PSUM bank
HP, WP = H + 2, W + 2  # padded image dims
F32 = mybir.dt.float32

DTYPE = "fp16"  # matmul operand dtype: "fp16", "bf16", or "f32r"


def _mm_dt():
    return {
        "fp16": mybir.dt.float16,
        "bf16": mybir.dt.bfloat16,
        "f32r": mybir.dt.float32r,
    }[DTYPE]


def _build_bands(w3x3: np.ndarray) -> np.ndarray:
    """[3, 128, 128] f32 banded lhsT: B[j][k, m] = w3x3[k - m, j]."""
    bands = np.zeros((3, P, P), np.float32)
    for j in range(3):
        for d in range(3):
            for m in range(MI):
                if m + d < P:
                    bands[j, m + d, m] = w3x3[d, j]
    return bands


def _matmuls(nc, ps, bt, at, at_col0, K):
    """3 column-shifted banded matmuls per 512-wide PSUM bank of ps."""
    nbank = ps.shape[1] // BANK
    for b in range(nbank):
        c0 = BANK * b
        for i, j in enumerate((1, 0, 2)):
            nc.tensor.matmul(
                ps[:, c0 : c0 + BANK],
                bt[:K, P * j : P * (j + 1)],
                at[:K, at_col0 + c0 + j : at_col0 + c0 + j + BANK],
                start=(i == 0),
                stop=(i == 2),
            )


def _gen_program():
    mmdt = _mm_dt()
    nc = bacc.Bacc("TRN2", target_bir_lowering=False, debug=False, num_devices=N)

    x = nc.dram_tensor("x", [C, HP, WP], F32, kind="ExternalInput")
    bands = nc.dram_tensor("bands", [3, P, P], mmdt, kind="ExternalInput")
    out = nc.dram_tensor("out", [C, H, W], F32, kind="ExternalOutput")

    with TileContext(nc) as tc:
        with (
            tc.tile_pool(name="consts", bufs=1) as cpool,
            tc.tile_pool(name="xin", bufs=4) as xpool,
            tc.tile_pool(name="xabs", bufs=4) as apool,
            tc.tile_pool(name="oev", bufs=4) as opool,
            tc.tile_pool(name="ps", bufs=3, space=MemorySpace.PSUM) as pspool,
        ):
            bt = cpool.tile([P, 3 * P], mmdt)
            for j in range(3):
                nc.sync.dma_start(out=bt[:, P * j : P * (j + 1)], in_=bands[j])

            for c in range(C):
                for q in range(2):  # quads of 4 row-tiles: t = 4q + k
                    r0 = 504 * q  # padded row of chunk 0
                    xt = xpool.tile([P, 4 * WP], F32)
                    src = AP(
                        x, c * HP * WP + r0 * WP,
                        [[WP, P], [MI * WP, 4], [1, WP]],
                    )
                    nc.sync.dma_start(out=xt[:], in_=src)

                    at = apool.tile([P, 4 * WP], mmdt)
                    nc.scalar.activation(
                        at[:], xt[:], mybir.ActivationFunctionType.Abs
                    )

                    ot = opool.tile([P, 4 * W], F32)
                    for k in range(4):
                        ps = pspool.tile([P, W], F32)
                        _matmuls(nc, ps, bt, at, k * WP, P)
                        ev = nc.vector.tensor_copy if k % 2 else nc.scalar.copy
                        ev(ot[:MI, k * W : (k + 1) * W], ps[:MI])

                    dst = AP(
                        out, c * H * W + 4 * MI * q * W,
                        [[W, MI], [MI * W, 4], [1, W]],
                    )
                    stq = nc.gpsimd if (2 * c + q) % 2 == 0 else nc.scalar
                    stq.dma_start(out=dst, in_=ot[:MI, :])

                # tail: out rows 1008..1023 (M=16), padded rhs rows 1008..1025
                K8, M8 = 18, 16
                xt = xpool.tile([P, 4 * WP], F32)
                nc.sync.dma_start(out=xt[:K8, :WP], in_=x[c, 1008 : 1008 + K8])
                at = apool.tile([P, 4 * WP], mmdt)
                nc.scalar.activation(
                    at[:K8, :WP], xt[:K8, :WP], mybir.ActivationFunctionType.Abs
                )
                ps = pspool.tile([P, W], F32)
                _matmuls(nc, ps, bt, at, 0, K8)
                ot = opool.tile([P, 4 * W], F32)
                nc.vector.tensor_copy(ot[:M8, :W], ps[:M8])
                nc.gpsimd.dma_start(out=out[c, 8 * MI :], in_=ot[:M8, :W])

    nc.compile()
    return nc


_PROGRAM = None


def _get_program():
    global _PROGRAM
    if _PROGRAM is None:
        _PROGRAM = _gen_program()
    return _PROGRAM


def _run(x: np.ndarray, weight: np.ndarray, trace: bool = False, tmpdir=None):
    assert x.shape == (N, C, H, W), x.shape
    w3x3 = np.asarray(weight, np.float32)[0, 0]
    np_mmdt = mybir.dt.np(_mm_dt())
    bands = _build_bands(w3x3).astype(np_mmdt)

    xp = np.pad(np.asarray(x, np.float32), ((0, 0), (0, 0), (1, 1), (1, 1)))

    nc = _get_program()
    in_maps = [
        {"x": np.ascontiguousarray(xp[i]), "bands": bands} for i in range(N)
    ]
    res = run_bass_kernel_spmd(
        nc, in_maps, core_ids=list(range(N)), trace=trace, tmpdir=tmpdir
    )
    out = np.stack([res.results[i]["out"] for i in range(N)])
    return out, res


def kernel(x: np.ndarray, weight: np.ndarray) -> np.ndarray:
    out, _ = _run(np.asarray(x), np.asarray(weight))
    return out
